# revision 14
# baseline (speedup 1.0000x reference)
"""Multi-head self-attention (b=2, n=2048, emb=1024, heads=16) on 8 trn2 cores.

Sharding: core c = (b, hg) with b = c // 4, hg = c % 4. Data parallel over
batch, tensor parallel over head-groups (4 heads / 256 emb-cols per core).
Each core computes Q/K/V projections for its heads, full attention for its
heads, and a partial output projection ctx_hg @ Wo[:, hg_slice].T of shape
[2048, 1024] (stored f16). The host sums the 4 partials per batch and adds
the rank-1 bias term bv @ Wo.T + bo.

v2 design vs the fp16 baseline:
- QKV projections and the attn@V (ctx) matmul run in fp8e4 with DoubleRow
  perf mode: contraction of 256 (two 128-tiles) per instruction. Weights and
  x are host-quantized to fp8 after a x32 rescale (w sigma 0.03 -> 1.0) so
  values sit in e4m3's normal range; the rescale cancels in softmax (S is
  1024x, absorbed by the exp scale) and via a 1/32 factor in the ctx
  normalize. S stays fp16 (contraction is only d=64, DoubleRow inapplicable,
  and plain fp8 has no PE speed advantage).
- exp runs on ACT straight out of PSUM in 1024-wide instructions, emitting
  fp8e4 E with a fused e^-2 shift (e4m3 max finite is 240; the shift cancels
  in normalization). A tunable subset of tiles can be offloaded to DVE using
  a Schraudolph int16-bitcast exp (~1% rms, softmax-averaged down ~30x);
  those tiles take an fp16 ctx path.
- x is loaded once (not per head-pair); all matmuls use 512-col moving
  operands; out-projection accumulates both head-pair passes in PSUM
  (start/stop) so only one copy per tile survives, on GpSimd.
- softmax normalize: rowsum rides the ctx matmul as an extra leading ones
  column; 1/rowsum via reciprocal_approx_fast on a [1,512] tile, broadcast
  on GpSimd, one fused scale-multiply on DVE.
- deferred work units (projections, V tiles, out-proj parcels) live in a
  `pending` registry; attention consumers run any still-pending prerequisite
  at its exact deadline, so schedule pacing is a pure perf knob and can
  never reorder a read before its producing write.
"""

import os
import sys

for _p in ("/opt/trn_rl_repo", "/root/.axon_site/_ro/trn_rl_repo"):
    if os.path.isdir(_p) and _p not in sys.path:
        sys.path.append(_p)

import numpy as np
import ml_dtypes

import concourse.bass as bass  # noqa: F401
import concourse.mybir as mybir
import concourse.tile as tile
from concourse import bacc
from concourse.bass_utils import run_bass_kernel_spmd

B, N, EMB, HEADS, HD = 2, 2048, 1024, 16, 64
N_CORES = 8
TP = 4                      # head-group shards per batch
DQ = EMB // TP              # 256 emb-cols (4 heads) per core
WSCALE = 32.0               # host premultiplier on Wq/Wk/Wv (and bq/bk)
K_EXP = (HD ** -0.5) / (WSCALE * WSCALE)   # exp scale on S' = 1024*S
C_SHIFT = 0.0               # E fp16: max logit ~8.7 -> E<=6000, fits f16; no shift needed
NQ = 512                    # moving-dim chunk everywhere
NJ = N // NQ                # 4 n chunks
NTP = 8                     # t-pairs per (p, j, h): 16 nk chunks of 128
# (tp, h) pairs whose exp runs on DVE (Schraudolph int16) instead of ACT.
# Tunable ACT/DVE balance knob; the ctx matmul for those tiles is fp16.
OFFLOAD = frozenset()
# Schraudolph fp16 constants: i16 = round((k*S' + C) * 1477.32 + 15293.36)
SCHRAU_A = K_EXP * 1477.3195
SCHRAU_B = C_SHIFT * 1477.3195 + 15293.36

F32 = mybir.dt.float32
F16 = mybir.dt.float16
F8 = mybir.dt.float8e4
I16 = mybir.dt.int16
FP = mybir.ActivationFunctionType
DR = mybir.MatmulPerfMode.DoubleRow


def build_program():
    nc = bacc.Bacc("TRN2", target_bir_lowering=False, debug=False,
                   num_devices=N_CORES)

    xT16 = nc.dram_tensor("xT16", [EMB, N], F16, kind="ExternalInput").ap()
    wq16 = nc.dram_tensor("wq16", [EMB, DQ], F16, kind="ExternalInput").ap()
    wk16 = nc.dram_tensor("wk16", [EMB, DQ], F16, kind="ExternalInput").ap()
    wv16 = nc.dram_tensor("wv16", [EMB, DQ], F16, kind="ExternalInput").ap()
    woT = nc.dram_tensor("woT", [DQ, EMB], F16, kind="ExternalInput").ap()
    bqd = nc.dram_tensor("bq_s", [DQ], F32, kind="ExternalInput").ap()
    bkd = nc.dram_tensor("bk_s", [DQ], F32, kind="ExternalInput").ap()
    out_part = nc.dram_tensor("out_part", [N, EMB], F16,
                              kind="ExternalOutput").ap()

    add, mult = mybir.AluOpType.add, mybir.AluOpType.mult

    with tile.TileContext(nc) as tc:
        with (
            tc.tile_pool(name="const", bufs=1) as const,
            tc.tile_pool(name="persist", bufs=1) as persist,
            tc.tile_pool(name="epool", bufs=2) as epool,
            tc.tile_pool(name="npool", bufs=2) as npool,
            tc.tile_pool(name="opool", bufs=3) as opool,
            # PSUM (8 banks): pp 2x1 + sa 2 + sb 2 + c0 1 + c1 1
            tc.tile_pool(name="ppool", bufs=2, space="PSUM") as ppool,
            tc.tile_pool(name="spool", bufs=1, space="PSUM") as spool,
            tc.tile_pool(name="cpool", bufs=1, space="PSUM") as cpool,
        ):
            # ---- weights / x (fp8, DoubleRow layout [pr, two]) ----
            wk_sb = const.tile([128, 8, DQ], F16, tag="wk")
            wq_sb = const.tile([128, 8, DQ], F16, tag="wq")
            wv_sb = const.tile([128, 8, DQ], F16, tag="wv")
            x16_sb = const.tile([128, 8, N], F16, tag="x16")
            wo_sb = const.tile([128, 2, EMB], F16, tag="wo")
            bq_sb = const.tile([128, 2], F32, tag="bq")
            bk_sb = const.tile([128, 2], F32, tag="bk")
            cshift = const.tile([128, 1], F32, tag="cshift")
            nc.vector.memset(cshift, C_SHIFT)

            x16in = xT16.rearrange("(k p) n -> p k n", p=128)
            wkin = wk16.rearrange("(k p) d -> k p d", p=128)
            for k in range(2):
                nc.sync.dma_start(out=wk_sb[:, k, :], in_=wkin[k])
                nc.sync.dma_start(out=x16_sb[:, k, 0:NQ],
                                  in_=x16in[:, k, 0:NQ])
            nc.sync.dma_start(out=bk_sb, in_=bkd.rearrange("(m p) -> p m", p=128))
            nc.sync.dma_start(out=wq_sb,
                              in_=wq16.rearrange("(k p) d -> p k d", p=128))
            nc.sync.dma_start(out=bq_sb, in_=bqd.rearrange("(m p) -> p m", p=128))
            for k in range(2, 8):
                nc.sync.dma_start(out=wk_sb[:, k, :], in_=wkin[k])
                nc.sync.dma_start(out=x16_sb[:, k, 0:NQ],
                                  in_=x16in[:, k, 0:NQ])
            nc.sync.dma_start(out=wv_sb,
                              in_=wv16.rearrange("(k p) d -> p k d", p=128))
            nc.sync.dma_start(out=x16_sb[:, :, NQ:N], in_=x16in[:, :, NQ:N])

            # ---- persistent activations ----
            qT = [persist.tile([128, N], F16, tag=f"qT{p}", name=f"qT{p}") for p in range(2)]
            kT = [persist.tile([128, N], F16, tag=f"kT{p}", name=f"kT{p}") for p in range(2)]
            ctxT = [persist.tile([128, N], F16, tag=f"ctxT{p}", name=f"ctxT{p}") for p in range(2)]
            # V augmented with a trailing ones col per head slot (rowsum
            # rides the ctx matmul as out partition 64)
            v16 = persist.tile([128, 2 * NTP, 4, HD + 1], F16, tag="v16")
            nc.vector.memset(v16[:, :, :, 64], 1.0)

            # ---- deferred work units ----
            def kq_group(p, n, wsb, bsb, dst):
                ps = ppool.tile([128, NQ], F32, tag="pp", name="kqp")
                for k in range(8):
                    nc.tensor.matmul(
                        ps, wsb[:, k, p * 128:(p + 1) * 128],
                        x16_sb[:, k, n * NQ:(n + 1) * NQ],
                        start=(k == 0), stop=(k == 7))
                nc.vector.tensor_tensor(
                    out=dst[p][:, n * NQ:(n + 1) * NQ], in0=ps,
                    in1=bsb[:, p:p + 1].broadcast_to([128, NQ]), op=add)

            def v_group(t):
                ps = ppool.tile([128, NQ], F32, tag="pp", name="vp")
                for k in range(8):
                    nc.tensor.matmul(
                        ps[:, 0:256], x16_sb[:, k, t * 128:(t + 1) * 128],
                        wv_sb[:, k, :],
                        start=(k == 0), stop=(k == 7))
                nc.vector.tensor_copy(
                    out=v16[:, t, :, 0:64],
                    in_=ps[:, 0:256].rearrange("p (h c) -> p h c", c=64))

            o_tiles = {}

            def oproj(m, eo):
                need(("wo",))
                po = ppool.tile([128, NQ], F32, tag="pp", name="po")
                for kp in range(2):
                    nc.tensor.matmul(
                        po, ctxT[kp][:, m * 128:(m + 1) * 128],
                        wo_sb[:, kp, eo * NQ:(eo + 1) * NQ],
                        start=(kp == 0), stop=(kp == 1))
                if eo == 0:
                    o_tiles[m] = opool.tile([128, EMB], F16, tag="o", name="o")
                o = o_tiles[m]
                nc.vector.tensor_copy(out=o[:, eo * NQ:(eo + 1) * NQ], in_=po)
                if eo == 1:
                    nc.sync.dma_start(
                        out=out_part[m * 128:(m + 1) * 128, :],
                        in_=o_tiles.pop(m))

            # pending registry: key -> closure. need() runs a unit at its
            # deadline if pacing hasn't already; the deque front-loads.
            pending = {}
            fillers = []

            def register(key, fn):
                pending[key] = fn
                fillers.append(key)

            def need(key):
                fn = pending.pop(key, None)
                if fn is not None:
                    fn()

            def pop_filler():
                while fillers:
                    key = fillers.pop(0)
                    if key in pending:
                        need(key)
                        return True
                return False

            register(("wo",), lambda: nc.sync.dma_start(
                out=wo_sb, in_=woT.rearrange("(kp p) e -> p kp e", p=128)))
            for p in range(2):
                for n in range(NJ):
                    register(("k", p, n),
                             lambda p=p, n=n: kq_group(p, n, wk_sb, bk_sb, kT))
                    register(("q", p, n),
                             lambda p=p, n=n: kq_group(p, n, wq_sb, bq_sb, qT))
            for t in range(16):
                register(("v", t), lambda t=t: v_group(t))

            # prefix: just enough for (p0, j0) to start streaming
            need(("k", 0, 0))
            need(("q", 0, 0))
            need(("v", 0))
            need(("v", 1))

            # ---- attention ----
            def attention(p, j, budget):
                cps = [cpool.tile([HD + 1, NQ], F32, tag=f"c{h}",
                                  name=f"c{h}") for h in range(2)]

                def s_mms(h, tp, tag):
                    need(("k", p, tp // 2))
                    st = spool.tile([128, 2, NQ], F32, tag=tag, name=tag)
                    lo = h * 64
                    for two in range(2):
                        t = 2 * tp + two
                        nc.tensor.matmul(
                            st[:, two, :],
                            kT[p][lo:lo + 64, t * 128:(t + 1) * 128],
                            qT[p][lo:lo + 64, j * NQ:(j + 1) * NQ],
                            start=True, stop=True)
                    return st

                def exp_tile(st, h, tp, tag):
                    if (tp, h) in OFFLOAD:
                        f = epool.tile([128, 2, NQ], F16, tag=f"f{tag}",
                                       name=f"f{tag}")
                        nc.vector.tensor_scalar(
                            out=f.bitcast(I16), in0=st,
                            scalar1=SCHRAU_A, scalar2=SCHRAU_B,
                            op0=mult, op1=add)
                        return f, True
                    e = epool.tile([128, 2, NQ], F16, tag=f"e{tag}",
                                   name=f"e{tag}")
                    nc.scalar.activation(e, st, FP.Exp,
                                         bias=cshift, scale=K_EXP)
                    return e, False

                def ctx_mms(e, is16, h, tp):
                    need(("v", 2 * tp))
                    need(("v", 2 * tp + 1))
                    hl = 2 * p + h
                    for two in range(2):
                        t = 2 * tp + two
                        nc.tensor.matmul(
                            cps[h], v16[:, t, hl], e[:, two, :],
                            start=(tp == 0 and two == 0),
                            stop=(tp == NTP - 1 and two == 1))

                def normalize(h):
                    rs = npool.tile([1, NQ], F32, tag="rs", name="rs")
                    nc.vector.tensor_copy(rs, cps[h][64:65, :])
                    rc = npool.tile([1, NQ], F32, tag="rc", name="rc")
                    nc.vector.reciprocal_approx_fast(rc, rs)
                    rb = npool.tile([64, NQ], F32, tag="rb", name="rb")
                    nc.gpsimd.partition_broadcast(rb, rc)
                    nc.vector.scalar_tensor_tensor(
                        out=ctxT[p][h * 64:(h + 1) * 64, j * NQ:(j + 1) * NQ],
                        in0=cps[h][0:64, :], scalar=1.0 / WSCALE,
                        in1=rb, op0=mult, op1=mult)

                need(("q", p, j))
                # heads sequential (the c-bank normalize chain hides under
                # the other head's window); ctx trails s/exp by one item.
                work = [(h, tp) for h in range(2) for tp in range(NTP)]
                prev = None
                popped = 0
                for wi, (h, tp) in enumerate(work):
                    st = s_mms(h, tp, "sa" if wi % 2 == 0 else "sb")
                    cur = (*exp_tile(st, h, tp, wi % 2), h, tp)
                    if prev is not None:
                        ctx_mms(*prev)
                        if prev[2] != h:       # crossed head boundary
                            normalize(prev[2])
                    prev = cur
                    while popped < (wi + 1) * budget / 16.0:
                        if not pop_filler():
                            popped = budget
                            break
                        popped += 1
                ctx_mms(*prev)
                normalize(prev[2])
                if j + 1 < NJ:
                    need(("q", p, j + 1))
                elif p == 0:
                    need(("k", 1, 0))
                    need(("q", 1, 0))

            for p in range(2):
                for j in range(NJ):
                    # pacing budget: drain everything queued so far by the
                    # end of this pair (correctness is need()-guarded anyway)
                    live = sum(1 for k in fillers if k in pending)
                    budget = -(-live // (NJ - j))
                    attention(p, j, budget)
                    if p == 1:
                        for m in range(4 * j, 4 * j + 4):
                            register(("o", m, 0), lambda m=m: oproj(m, 0))
                            register(("o", m, 1), lambda m=m: oproj(m, 1))
            while pop_filler():
                pass

    nc.compile()
    return nc


_NC_CACHE = {}


def _get_program():
    if "nc" not in _NC_CACHE:
        _NC_CACHE["nc"] = build_program()
    return _NC_CACHE["nc"]


FP8 = ml_dtypes.float8_e4m3


def make_in_maps(x, Wq, bq, Wk, bk, Wv, bv, Wo, bo):
    x = np.asarray(x, np.float32)
    xT16s = [np.ascontiguousarray(x[b].T).astype(np.float16) for b in range(B)]
    in_maps = []
    for c in range(N_CORES):
        b, hg = divmod(c, TP)
        sl = slice(hg * DQ, (hg + 1) * DQ)
        in_maps.append({
            "xT16": xT16s[b],
            "wq16": np.ascontiguousarray(
                (WSCALE * np.asarray(Wq, np.float32))[sl, :].T).astype(
                    np.float16),
            "wk16": np.ascontiguousarray(
                (WSCALE * np.asarray(Wk, np.float32))[sl, :].T).astype(
                    np.float16),
            "wv16": np.ascontiguousarray(
                (WSCALE * np.asarray(Wv, np.float32))[sl, :].T).astype(
                    np.float16),
            "woT": np.ascontiguousarray(
                np.asarray(Wo, np.float16)[:, sl].T),
            "bq_s": np.ascontiguousarray(
                WSCALE * np.asarray(bq, np.float32)[sl]),
            "bk_s": np.ascontiguousarray(
                WSCALE * np.asarray(bk, np.float32)[sl]),
        })
    return in_maps


def assemble_output(results, Wv_bias_term):
    out = np.empty((B, N, EMB), np.float32)
    for b in range(B):
        acc = results[b * TP]["out_part"].astype(np.float32)
        for g in range(1, TP):
            acc += results[b * TP + g]["out_part"].astype(np.float32)
        out[b] = acc + Wv_bias_term
    return out


def kernel(x, Wq, bq, Wk, bk, Wv, bv, Wo, bo):
    nc = _get_program()
    in_maps = make_in_maps(x, Wq, bq, Wk, bk, Wv, bv, Wo, bo)
    res = run_bass_kernel_spmd(nc, in_maps, list(range(N_CORES)))
    bias_term = (np.asarray(bv, np.float32) @ np.asarray(Wo, np.float32).T
                 + np.asarray(bo, np.float32))
    return assemble_output(res.results, bias_term)


# revision 16
# speedup vs baseline: 1.0579x; 1.0579x over previous
"""Multi-head self-attention (b=2, n=2048, emb=1024, heads=16) on 8 trn2 cores.

Sharding: core c = (b, hg) with b = c // 4, hg = c % 4. Data parallel over
batch, tensor parallel over head-groups (4 heads / 256 emb-cols per core).
Each core computes Q/K/V projections for its heads, full attention for its
heads, and a partial output projection ctx_hg @ Wo[:, hg_slice].T of shape
[2048, 1024] (stored f16). The host sums the 4 partials per batch and adds
the rank-1 bias term bv @ Wo.T + bo.

v2 design vs the fp16 baseline:
- QKV projections and the attn@V (ctx) matmul run in fp8e4 with DoubleRow
  perf mode: contraction of 256 (two 128-tiles) per instruction. Weights and
  x are host-quantized to fp8 after a x32 rescale (w sigma 0.03 -> 1.0) so
  values sit in e4m3's normal range; the rescale cancels in softmax (S is
  1024x, absorbed by the exp scale) and via a 1/32 factor in the ctx
  normalize. S stays fp16 (contraction is only d=64, DoubleRow inapplicable,
  and plain fp8 has no PE speed advantage).
- exp runs on ACT straight out of PSUM in 1024-wide instructions, emitting
  fp8e4 E with a fused e^-2 shift (e4m3 max finite is 240; the shift cancels
  in normalization). A tunable subset of tiles can be offloaded to DVE using
  a Schraudolph int16-bitcast exp (~1% rms, softmax-averaged down ~30x);
  those tiles take an fp16 ctx path.
- x is loaded once (not per head-pair); all matmuls use 512-col moving
  operands; out-projection accumulates both head-pair passes in PSUM
  (start/stop) so only one copy per tile survives, on GpSimd.
- softmax normalize: rowsum rides the ctx matmul as an extra leading ones
  column; 1/rowsum via reciprocal_approx_fast on a [1,512] tile, broadcast
  on GpSimd, one fused scale-multiply on DVE.
- deferred work units (projections, V tiles, out-proj parcels) live in a
  `pending` registry; attention consumers run any still-pending prerequisite
  at its exact deadline, so schedule pacing is a pure perf knob and can
  never reorder a read before its producing write.
"""

import os
import sys

for _p in ("/opt/trn_rl_repo", "/root/.axon_site/_ro/trn_rl_repo"):
    if os.path.isdir(_p) and _p not in sys.path:
        sys.path.append(_p)

import numpy as np
import ml_dtypes

import concourse.bass as bass  # noqa: F401
import concourse.mybir as mybir
import concourse.tile as tile
from concourse import bacc
from concourse.bass_utils import run_bass_kernel_spmd

B, N, EMB, HEADS, HD = 2, 2048, 1024, 16, 64
N_CORES = 8
TP = 4                      # head-group shards per batch
DQ = EMB // TP              # 256 emb-cols (4 heads) per core
WSCALE = 32.0               # host premultiplier on Wq/Wk/Wv (and bq/bk)
K_EXP = (HD ** -0.5) / (WSCALE * WSCALE)   # exp scale on S' = 1024*S
C_SHIFT = 0.0               # E fp16: max logit ~8.7 -> E<=6000, fits f16; no shift needed
NQ = 512                    # moving-dim chunk everywhere
NJ = N // NQ                # 4 n chunks
NTP = 8                     # t-pairs per (p, j, h): 16 nk chunks of 128
# (tp, h) pairs whose exp runs on DVE (Schraudolph int16) instead of ACT.
# Tunable ACT/DVE balance knob; the ctx matmul for those tiles is fp16.
OFFLOAD = frozenset()
# Schraudolph fp16 constants: i16 = round((k*S' + C) * 1477.32 + 15293.36)
SCHRAU_A = K_EXP * 1477.3195
SCHRAU_B = C_SHIFT * 1477.3195 + 15293.36

F32 = mybir.dt.float32
F16 = mybir.dt.float16
F8 = mybir.dt.float8e4
I16 = mybir.dt.int16
FP = mybir.ActivationFunctionType
DR = mybir.MatmulPerfMode.DoubleRow


def build_program():
    nc = bacc.Bacc("TRN2", target_bir_lowering=False, debug=False,
                   num_devices=N_CORES)

    xT16 = nc.dram_tensor("xT16", [EMB, N], F16, kind="ExternalInput").ap()
    wq16 = nc.dram_tensor("wq16", [EMB, DQ], F16, kind="ExternalInput").ap()
    wk16 = nc.dram_tensor("wk16", [EMB, DQ], F16, kind="ExternalInput").ap()
    wv16 = nc.dram_tensor("wv16", [EMB, DQ], F16, kind="ExternalInput").ap()
    woT = nc.dram_tensor("woT", [DQ, EMB], F16, kind="ExternalInput").ap()
    bqd = nc.dram_tensor("bq_s", [DQ], F32, kind="ExternalInput").ap()
    bkd = nc.dram_tensor("bk_s", [DQ], F32, kind="ExternalInput").ap()
    out_part = nc.dram_tensor("out_part", [N, EMB], F16,
                              kind="ExternalOutput").ap()

    add, mult = mybir.AluOpType.add, mybir.AluOpType.mult

    with tile.TileContext(nc) as tc:
        with (
            tc.tile_pool(name="const", bufs=1) as const,
            tc.tile_pool(name="persist", bufs=1) as persist,
            tc.tile_pool(name="epool", bufs=5) as epool,
            tc.tile_pool(name="npool", bufs=2) as npool,
            tc.tile_pool(name="opool", bufs=3) as opool,
            # PSUM (8 banks): pp 2x1 + sa 2 + sb 2 + c0 1 + c1 1
            tc.tile_pool(name="ppool", bufs=2, space="PSUM") as ppool,
            tc.tile_pool(name="spool", bufs=1, space="PSUM") as spool,
            tc.tile_pool(name="cpool", bufs=1, space="PSUM") as cpool,
        ):
            # ---- weights / x (fp8, DoubleRow layout [pr, two]) ----
            wk_sb = const.tile([128, 8, DQ], F16, tag="wk")
            wq_sb = const.tile([128, 8, DQ], F16, tag="wq")
            wv_sb = const.tile([128, 8, DQ], F16, tag="wv")
            x16_sb = const.tile([128, 8, N], F16, tag="x16")
            wo_sb = const.tile([128, 2, EMB], F16, tag="wo")
            bq_sb = const.tile([128, 2], F32, tag="bq")
            bk_sb = const.tile([128, 2], F32, tag="bk")
            cshift = const.tile([128, 1], F32, tag="cshift")
            nc.vector.memset(cshift, C_SHIFT)

            x16in = xT16.rearrange("(k p) n -> p k n", p=128)
            nc.sync.dma_start(out=wk_sb,
                              in_=wk16.rearrange("(k p) d -> p k d", p=128))
            nc.sync.dma_start(out=x16_sb[:, :, 0:NQ], in_=x16in[:, :, 0:NQ])
            nc.sync.dma_start(out=wq_sb,
                              in_=wq16.rearrange("(k p) d -> p k d", p=128))
            nc.sync.dma_start(out=wv_sb,
                              in_=wv16.rearrange("(k p) d -> p k d", p=128))
            nc.sync.dma_start(out=bq_sb, in_=bqd.rearrange("(m p) -> p m", p=128))
            nc.sync.dma_start(out=bk_sb, in_=bkd.rearrange("(m p) -> p m", p=128))
            for ns in range(1, NJ):
                nc.sync.dma_start(out=x16_sb[:, :, ns * NQ:(ns + 1) * NQ],
                                  in_=x16in[:, :, ns * NQ:(ns + 1) * NQ])

            # ---- persistent activations ----
            qT = [persist.tile([128, N], F16, tag=f"qT{p}", name=f"qT{p}") for p in range(2)]
            kT = [persist.tile([128, N], F16, tag=f"kT{p}", name=f"kT{p}") for p in range(2)]
            ctxT = [persist.tile([128, N], F16, tag=f"ctxT{p}", name=f"ctxT{p}") for p in range(2)]
            # V augmented with a trailing ones col per head slot (rowsum
            # rides the ctx matmul as out partition 64)
            v16 = persist.tile([128, 2 * NTP, 4, HD + 1], F16, tag="v16")
            nc.vector.memset(v16[:, :, :, 64], 1.0)

            # ---- deferred work units ----
            def kq_group(p, n, wsb, bsb, dst):
                ps = ppool.tile([128, NQ], F32, tag="pp", name="kqp")
                for k in range(8):
                    nc.tensor.matmul(
                        ps, wsb[:, k, p * 128:(p + 1) * 128],
                        x16_sb[:, k, n * NQ:(n + 1) * NQ],
                        start=(k == 0), stop=(k == 7))
                nc.vector.tensor_tensor(
                    out=dst[p][:, n * NQ:(n + 1) * NQ], in0=ps,
                    in1=bsb[:, p:p + 1].broadcast_to([128, NQ]), op=add)

            def v_group(t):
                ps = ppool.tile([128, NQ], F32, tag="pp", name="vp")
                for k in range(8):
                    nc.tensor.matmul(
                        ps[:, 0:256], x16_sb[:, k, t * 128:(t + 1) * 128],
                        wv_sb[:, k, :],
                        start=(k == 0), stop=(k == 7))
                nc.vector.tensor_copy(
                    out=v16[:, t, :, 0:64],
                    in_=ps[:, 0:256].rearrange("p (h c) -> p h c", c=64))

            o_tiles = {}

            def oproj(m, eo):
                need(("wo",))
                po = ppool.tile([128, NQ], F32, tag="pp", name="po")
                for kp in range(2):
                    nc.tensor.matmul(
                        po, ctxT[kp][:, m * 128:(m + 1) * 128],
                        wo_sb[:, kp, eo * NQ:(eo + 1) * NQ],
                        start=(kp == 0), stop=(kp == 1))
                if eo == 0:
                    o_tiles[m] = opool.tile([128, EMB], F16, tag="o", name="o")
                o = o_tiles[m]
                nc.vector.tensor_copy(out=o[:, eo * NQ:(eo + 1) * NQ], in_=po)
                if eo == 1:
                    nc.sync.dma_start(
                        out=out_part[m * 128:(m + 1) * 128, :],
                        in_=o_tiles.pop(m))

            # pending registry: key -> closure. need() runs a unit at its
            # deadline if pacing hasn't already; the deque front-loads.
            pending = {}
            fillers = []

            def register(key, fn):
                pending[key] = fn
                fillers.append(key)

            def need(key):
                fn = pending.pop(key, None)
                if fn is not None:
                    fn()

            def pop_filler():
                while fillers:
                    key = fillers.pop(0)
                    if key in pending:
                        need(key)
                        return True
                return False

            register(("wo",), lambda: nc.sync.dma_start(
                out=wo_sb, in_=woT.rearrange("(kp p) e -> p kp e", p=128)))
            for p in range(2):
                for n in range(NJ):
                    register(("k", p, n),
                             lambda p=p, n=n: kq_group(p, n, wk_sb, bk_sb, kT))
                    register(("q", p, n),
                             lambda p=p, n=n: kq_group(p, n, wq_sb, bq_sb, qT))
            for t in range(16):
                register(("v", t), lambda t=t: v_group(t))

            # prefix: just enough for (p0, j0) to start streaming
            need(("k", 0, 0))
            need(("q", 0, 0))
            need(("v", 0))
            need(("v", 1))

            # ---- attention ----
            def attention(p, j, budget):
                cps = [cpool.tile([HD + 1, NQ], F32, tag=f"c{h}",
                                  name=f"c{h}") for h in range(2)]

                def s_mms(h, tp, tag):
                    need(("k", p, tp // 2))
                    st = spool.tile([128, 2, NQ], F32, tag=tag, name=tag)
                    lo = h * 64
                    for two in range(2):
                        t = 2 * tp + two
                        nc.tensor.matmul(
                            st[:, two, :],
                            kT[p][lo:lo + 64, t * 128:(t + 1) * 128],
                            qT[p][lo:lo + 64, j * NQ:(j + 1) * NQ],
                            start=True, stop=True)
                    return st

                def exp_tile(st, h, tp, tag):
                    if (tp, h) in OFFLOAD:
                        f = epool.tile([128, 2, NQ], F16, tag=f"f{tag}",
                                       name=f"f{tag}")
                        nc.vector.tensor_scalar(
                            out=f.bitcast(I16), in0=st,
                            scalar1=SCHRAU_A, scalar2=SCHRAU_B,
                            op0=mult, op1=add)
                        return f, True
                    e = epool.tile([128, 2, NQ], F16, tag=f"e{tag}",
                                   name=f"e{tag}")
                    nc.scalar.activation(e, st, FP.Exp,
                                         bias=cshift, scale=K_EXP)
                    return e, False

                def ctx_mms(e, is16, h, tp):
                    need(("v", 2 * tp))
                    need(("v", 2 * tp + 1))
                    hl = 2 * p + h
                    for two in range(2):
                        t = 2 * tp + two
                        nc.tensor.matmul(
                            cps[h], v16[:, t, hl], e[:, two, :],
                            start=(tp == 0 and two == 0),
                            stop=(tp == NTP - 1 and two == 1))

                def normalize(h):
                    rs = npool.tile([1, NQ], F32, tag="rs", name="rs")
                    nc.vector.tensor_copy(rs, cps[h][64:65, :])
                    rc = npool.tile([1, NQ], F32, tag="rc", name="rc")
                    nc.vector.reciprocal_approx_fast(rc, rs)
                    rb = npool.tile([64, NQ], F32, tag="rb", name="rb")
                    nc.gpsimd.partition_broadcast(rb, rc)
                    nc.vector.scalar_tensor_tensor(
                        out=ctxT[p][h * 64:(h + 1) * 64, j * NQ:(j + 1) * NQ],
                        in0=cps[h][0:64, :], scalar=1.0 / WSCALE,
                        in1=rb, op0=mult, op1=mult)

                need(("q", p, j))
                # heads sequential (the c-bank normalize chain hides under
                # the other head's window); ctx trails s/exp by one item,
                # except the first window where it trails 8 so the exp
                # stream saturates ACT while the V tiles are still building.
                from collections import deque as _dq
                trail = 8 if (p == 0 and j == 0) else 1
                work = [(h, tp) for h in range(2) for tp in range(NTP)]
                pend = _dq()
                popped = 0

                def drain_one():
                    it = pend.popleft()
                    ctx_mms(*it)
                    if it[3] == NTP - 1:       # last tile of its head
                        normalize(it[2])

                for wi, (h, tp) in enumerate(work):
                    st = s_mms(h, tp, "sa" if wi % 2 == 0 else "sb")
                    pend.append((*exp_tile(st, h, tp, wi % 2), h, tp))
                    if len(pend) > trail:
                        drain_one()
                    while popped < (wi + 1) * budget / 16.0:
                        if not pop_filler():
                            popped = budget
                            break
                        popped += 1
                while pend:
                    drain_one()
                if j + 1 < NJ:
                    need(("q", p, j + 1))
                elif p == 0:
                    need(("k", 1, 0))
                    need(("q", 1, 0))

            for p in range(2):
                for j in range(NJ):
                    # pacing budget: drain everything queued so far by the
                    # end of this pair (correctness is need()-guarded anyway)
                    live = sum(1 for k in fillers if k in pending)
                    budget = -(-live // (NJ - j))
                    attention(p, j, budget)
                    if p == 1:
                        for m in range(4 * j, 4 * j + 4):
                            register(("o", m, 0), lambda m=m: oproj(m, 0))
                            register(("o", m, 1), lambda m=m: oproj(m, 1))
            while pop_filler():
                pass

    nc.compile()
    return nc


_NC_CACHE = {}


def _get_program():
    if "nc" not in _NC_CACHE:
        _NC_CACHE["nc"] = build_program()
    return _NC_CACHE["nc"]


FP8 = ml_dtypes.float8_e4m3


def make_in_maps(x, Wq, bq, Wk, bk, Wv, bv, Wo, bo):
    x = np.asarray(x, np.float32)
    xT16s = [np.ascontiguousarray(x[b].T).astype(np.float16) for b in range(B)]
    in_maps = []
    for c in range(N_CORES):
        b, hg = divmod(c, TP)
        sl = slice(hg * DQ, (hg + 1) * DQ)
        in_maps.append({
            "xT16": xT16s[b],
            "wq16": np.ascontiguousarray(
                (WSCALE * np.asarray(Wq, np.float32))[sl, :].T).astype(
                    np.float16),
            "wk16": np.ascontiguousarray(
                (WSCALE * np.asarray(Wk, np.float32))[sl, :].T).astype(
                    np.float16),
            "wv16": np.ascontiguousarray(
                (WSCALE * np.asarray(Wv, np.float32))[sl, :].T).astype(
                    np.float16),
            "woT": np.ascontiguousarray(
                np.asarray(Wo, np.float16)[:, sl].T),
            "bq_s": np.ascontiguousarray(
                WSCALE * np.asarray(bq, np.float32)[sl]),
            "bk_s": np.ascontiguousarray(
                WSCALE * np.asarray(bk, np.float32)[sl]),
        })
    return in_maps


def assemble_output(results, Wv_bias_term):
    out = np.empty((B, N, EMB), np.float32)
    for b in range(B):
        acc = results[b * TP]["out_part"].astype(np.float32)
        for g in range(1, TP):
            acc += results[b * TP + g]["out_part"].astype(np.float32)
        out[b] = acc + Wv_bias_term
    return out


def kernel(x, Wq, bq, Wk, bk, Wv, bv, Wo, bo):
    nc = _get_program()
    in_maps = make_in_maps(x, Wq, bq, Wk, bk, Wv, bv, Wo, bo)
    res = run_bass_kernel_spmd(nc, in_maps, list(range(N_CORES)))
    bias_term = (np.asarray(bv, np.float32) @ np.asarray(Wo, np.float32).T
                 + np.asarray(bo, np.float32))
    return assemble_output(res.results, bias_term)
